# revision 62
# baseline (speedup 1.0000x reference)
"""Fused masked-attention kernel for Trainium2, data-parallel over batch on 8 cores.

v26 design notes (all per core; one batch element per core):
- Steady state per tile (chunk pair): score-pair matmuls (row-split K=64
  concurrent), ACT exp (the wall, ~1.13us/tile, 145us total), DVE mask
  multiply, serial AV accumulation in PSUM (ps_o double-buffered across
  q-blocks). Minimal-energy design: measured power throttling (activity_1
  caps engine util at 50%) punishes extra engine work, so no Pool casts and
  no PE-side mask matmuls in steady state.
- Two-pass chunk sweep to fix the DMA-supply-bound ramp: pass 1 covers
  k-chunks 0:24 for all 8 q-blocks (needs only K0-5/V0-5 resident), pass 2
  covers chunks 24:32 (K6,7/V6,7 stream in near the end of pass 1). Each
  q-block's pass-1 AV partial parks in SBUF f32 and is added back during
  pass 2's epilogue. This keeps aggregate DMA demand under the measured
  ~430GB/s fleet rate instead of needing all of K+V+masks inside the first
  q-block.
- Masks: pass-1 q-blocks 0-3 ship fp8e4 {0,1} (DVE 1x multiply) to thin the
  earliest window; everything else f16 half-tiles (DVE 2x multiply),
  prefetched a q-block ahead in two pieces for fine-grained unblocking.
- K0,K1,q0 DMAs lead the queue; PE warmup (HAM) runs on a memset tile at
  t~0 with no DMA dependency, so projections start the moment K0 lands.
- Output ships unnormalized O^T+Z rows; host does divide+transpose in f64.
"""

import numpy as np
import ml_dtypes

import concourse.bass as bass
import concourse.tile as tile
from concourse import bacc, mybir
from concourse import bass_utils

B, L, E, H = 8, 4096, 1024, 64
NCORES = 8
F32 = mybir.dt.float32
F16 = mybir.dt.float16
F8 = mybir.dt.float8e4

LB = 512           # q-block and projection block width
NQB = L // LB      # 8
NCH = L // 128     # 32 k-chunks
NEC = E // 128     # 8 e-chunks
NG1 = 10           # tiles (chunk pairs) per q-block in pass 1 (chunks 0:20)
NG2 = 6            # tiles per q-block in pass 2 (chunks 20:32)
C1 = 2 * NG1       # 24
NQB8 = 4           # pass-1 q-blocks whose mask ships fp8


def build_nc():
    nc = bacc.Bacc(
        "TRN2",
        target_bir_lowering=False,
        debug=False,
        enable_asserts=False,
        num_devices=NCORES,
    )
    q2 = nc.dram_tensor("q2", [NQB, 128, NEC, LB], F16, kind="ExternalInput").ap()
    k2 = nc.dram_tensor("k2", [NQB, 128, NEC, LB], F16, kind="ExternalInput").ap()
    v2 = nc.dram_tensor("v2", [NQB, 128, NEC, LB], F16, kind="ExternalInput").ap()
    m8 = nc.dram_tensor("m8", [NQB8, 128, C1, LB], F8, kind="ExternalInput").ap()
    mu8 = nc.dram_tensor("mu8", [NQB, 128, NCH, LB], F16, kind="ExternalInput").ap()
    wqD = nc.dram_tensor("wqD", [E, 128], F16, kind="ExternalInput").ap()
    wkD = nc.dram_tensor("wkD", [E, 128], F16, kind="ExternalInput").ap()
    wvT = nc.dram_tensor("wvT", [E, H], F16, kind="ExternalInput").ap()
    ident = nc.dram_tensor("ident", [64, 64], F16, kind="ExternalInput").ap()
    out = nc.dram_tensor("out", [H + 1, L], F32, kind="ExternalOutput").ap()

    EXP = mybir.ActivationFunctionType.Exp

    with tile.TileContext(nc) as tc:
        with (
            tc.tile_pool(name="const", bufs=1) as constp,
            tc.tile_pool(name="persist", bufs=1) as persist,
            tc.tile_pool(name="kin", bufs=3) as kinp,
            tc.tile_pool(name="vin", bufs=2) as vinp,
            tc.tile_pool(name="qin", bufs=2) as qinp,
            tc.tile_pool(name="m8p", bufs=2) as m8pool,
            tc.tile_pool(name="mpk", bufs=3) as mpool,
            tc.tile_pool(name="mp2", bufs=2) as mpool2,
            tc.tile_pool(name="pt", bufs=6) as ptpool,
            tc.tile_pool(name="osb", bufs=2) as opool,
            tc.tile_pool(name="ps_st", bufs=2, space="PSUM") as ps_st,
            tc.tile_pool(name="ps_o", bufs=2, space="PSUM") as ps_o,
            tc.tile_pool(name="ps_pj", bufs=1, space="PSUM") as ps_pj,
        ):
            # ---- warmup constant, ready with no DMA dependency ----
            wu = constp.tile([128, 128], F16)
            nc.vector.memset(wu[:], 0.01)

            # ---- PE warmup (HAM) on the memset tile, t ~ 0 ----
            p_w = ps_st.tile([128, 1024], F32, tag="p_st")
            for w in range(80):
                nc.tensor.matmul(
                    p_w[:, 0:128], wu[:], wu[:], start=True, stop=True,
                )

            # ---- K0,K1,q0 lead the DMA queue (weights only gate the
            # projections, which start after K0 lands anyway) ----
            k_in0 = kinp.tile([128, NEC, LB], F16, tag="kin")
            nc.sync.dma_start(k_in0[:], k2[0])
            k_in1 = kinp.tile([128, NEC, LB], F16, tag="kin")
            nc.sync.dma_start(k_in1[:], k2[1])
            q0 = qinp.tile([128, NEC, LB], F16, tag="qin")
            nc.sync.dma_start(q0[:], q2[0])

            # ---- constants / weights ----
            wq_sb = constp.tile([128, NEC, 128], F16)
            wk_sb = constp.tile([128, NEC, 128], F16)
            wv_sb = constp.tile([128, NEC, H], F16)
            nc.sync.dma_start(wq_sb[:], wqD.rearrange("(c p) h -> p c h", p=128))
            nc.sync.dma_start(wk_sb[:], wkD.rearrange("(c p) h -> p c h", p=128))
            nc.sync.dma_start(wv_sb[:], wvT.rearrange("(c p) h -> p c h", p=128))
            id_sb = constp.tile([64, 64], F16)
            nc.sync.dma_start(id_sb[:], ident)

            # persistent projected tensors
            QT_sb = persist.tile([128, L], F16)   # rows 0:64 = Q^T, 64:128 copy
            KT_sb = persist.tile([128, L], F16)
            VT_sb = persist.tile([64, L], F16)    # V^T staging
            V_sb = persist.tile([128, NCH, 128], F16)  # [k, h] + ones col 64
            part_sb = persist.tile([H + 1, NQB, LB], F32)  # pass-1 AV partials
            nc.vector.memset(V_sb[:, :, H : 128], 0.0)
            nc.vector.memset(V_sb[:, :, H : H + 1], 1.0)

            def proj_k_block(b, k_in):
                ls = b * LB
                p_pj = ps_pj.tile([128, LB], F32, tag="pj")
                for ec in range(NEC):
                    nc.tensor.matmul(
                        p_pj[:], wk_sb[:, ec, :], k_in[:, ec, :],
                        start=(ec == 0), stop=(ec == NEC - 1),
                    )
                nc.vector.tensor_copy(KT_sb[:, ls : ls + LB], p_pj[:])

            def proj_q_block(b, q_in):
                ls = b * LB
                p_pj = ps_pj.tile([128, LB], F32, tag="pj")
                for ec in range(NEC):
                    nc.tensor.matmul(
                        p_pj[:], wq_sb[:, ec, :], q_in[:, ec, :],
                        start=(ec == 0), stop=(ec == NEC - 1),
                    )
                nc.vector.tensor_copy(QT_sb[:, ls : ls + LB], p_pj[:])

            def proj_v_block(b, v_in):
                ls = b * LB
                p_pj = ps_pj.tile([128, LB], F32, tag="pj")
                for ec in range(NEC):
                    nc.tensor.matmul(
                        p_pj[0:H, :], wv_sb[:, ec, :], v_in[:, ec, :],
                        start=(ec == 0), stop=(ec == NEC - 1),
                    )
                nc.vector.tensor_copy(VT_sb[:, ls : ls + LB], p_pj[0:H, :])
                for sub in range(4):
                    c = b * 4 + sub
                    p_tr = ps_pj.tile([128, H], F16, tag="pjt")
                    nc.tensor.transpose(
                        p_tr[:], VT_sb[:, c * 128 : (c + 1) * 128], id_sb[:]
                    )
                    nc.vector.tensor_copy(V_sb[:, c, 0:H], p_tr[:])

            def load_one(pool, tag, src):
                t = pool.tile([128, NEC, LB], F16, tag=tag)
                nc.sync.dma_start(t[:], src)
                return t

            def scores_and_exp(qb, cA):
                qs = qb * LB
                ps = ps_st.tile([128, 1024], F32, tag="p_st")
                nc.tensor.matmul(
                    ps[:, 0:512],
                    KT_sb[0:64, cA * 128 : (cA + 1) * 128],
                    QT_sb[0:64, qs : qs + LB],
                    start=True, stop=True,
                )
                nc.tensor.matmul(
                    ps[:, 512:1024],
                    KT_sb[64:128, (cA + 1) * 128 : (cA + 2) * 128],
                    QT_sb[64:128, qs : qs + LB],
                    start=True, stop=True,
                )
                return ps

            def exp_mask_av(ps, p_o, cA, msrc, first, last):
                pt = ptpool.tile([128, 1024], F16, tag="pt")
                nc.scalar.activation(pt[:], ps[:], EXP, scale=0.125)
                nc.vector.tensor_mul(
                    pt[:], pt[:], msrc.rearrange("p c q -> p (c q)")
                )
                nc.tensor.matmul(
                    p_o[:], V_sb[:, cA, :], pt[:, 0:512],
                    start=first, stop=False,
                )
                nc.tensor.matmul(
                    p_o[:], V_sb[:, cA + 1, :], pt[:, 512:1024],
                    start=False, stop=last,
                )

            # ---- startup: fp8 mask pieces interleaved with K2/V0/V1 ----
            m8_sb0 = m8pool.tile([128, C1, LB], F8, tag="m8")
            nc.sync.dma_start(m8_sb0[:, 0:4, :], m8[0, :, 0:4, :])
            k_in2 = load_one(kinp, "kin", k2[2])
            nc.sync.dma_start(m8_sb0[:, 4:12, :], m8[0, :, 4:12, :])
            v_in0 = load_one(vinp, "vin", v2[0])
            nc.sync.dma_start(m8_sb0[:, 12:C1, :], m8[0, :, 12:C1, :])
            v_in1 = load_one(vinp, "vin", v2[1])

            proj_k_block(0, k_in0)
            proj_k_block(1, k_in1)
            proj_q_block(0, q0)
            proj_v_block(0, v_in0)

            # ---- pass 1: chunks 0:24 for every q-block ----
            cur_mask = ("f8", m8_sb0)
            q_next = None
            m_next = None
            k_pend = {2: k_in2}
            v_pend = {1: v_in1}
            kv_tail = {}
            for qb in range(NQB):
                p_o = ps_o.tile([128, LB], F32, tag="p_o")
                for g in range(NG1):
                    cA = 2 * g
                    ps = scores_and_exp(qb, cA)
                    # streaming projections during qb0 (K0-5, V0-5)
                    if qb == 0:
                        if g % 2 == 0:
                            b = g // 2 + 2
                            if b <= 4:
                                proj_k_block(b, k_pend.pop(b))
                            if b + 1 <= 4:
                                k_pend[b + 1] = load_one(kinp, "kin", k2[b + 1])
                        else:
                            b = (g + 1) // 2
                            if b <= 4:
                                proj_v_block(b, v_pend.pop(b))
                            if b + 1 <= 4:
                                v_pend[b + 1] = load_one(vinp, "vin", v2[b + 1])
                    # K5-7/V5-7 stream near the end of pass 1
                    if qb == 6 and g in (2, 6):
                        kv_tail["k" + str(5 + (g == 6))] = load_one(
                            kinp, "kin", k2[5 + (g == 6)]
                        )
                    if qb == 7:
                        if g == 0:
                            kv_tail["k7"] = load_one(kinp, "kin", k2[7])
                            kv_tail["v5"] = load_one(vinp, "vin", v2[5])
                        if g == 1:
                            proj_k_block(5, kv_tail.pop("k5"))
                        if g == 2:
                            kv_tail["v6"] = load_one(vinp, "vin", v2[6])
                            mp2_t = mpool2.tile([128, NCH - C1, LB], F16, tag="mp2")
                            nc.sync.dma_start(mp2_t[:], mu8[0, :, C1:NCH, :])
                            kv_tail["mp2"] = mp2_t
                        if g == 3:
                            proj_k_block(6, kv_tail.pop("k6"))
                        if g == 4:
                            kv_tail["v7"] = load_one(vinp, "vin", v2[7])
                        if g == 5:
                            proj_v_block(5, kv_tail.pop("v5"))
                        if g == 6:
                            proj_k_block(7, kv_tail.pop("k7"))
                        if g == 7:
                            proj_v_block(6, kv_tail.pop("v6"))
                        if g == 9:
                            proj_v_block(7, kv_tail.pop("v7"))
                    # mask source for this tile
                    if cur_mask[0] == "f8":
                        msrc = cur_mask[1][:, cA : cA + 2, :]
                    else:
                        h = cur_mask[1][g // 5]
                        ch = cA - 10 * (g // 5)
                        msrc = h[:, ch : ch + 2, :]
                    exp_mask_av(ps, p_o, cA, msrc, g == 0, g == NG1 - 1)
                    # prefetch hooks for qb+1 (pass 1)
                    if qb + 1 < NQB:
                        if g == 3:
                            q_next = load_one(qinp, "qin", q2[qb + 1])
                        if g == 7:
                            proj_q_block(qb + 1, q_next)
                        if g == 6:
                            if qb + 1 < NQB8:
                                m8n = m8pool.tile([128, C1, LB], F8, tag="m8")
                                m_next = ("f8", m8n)
                                nc.sync.dma_start(
                                    m8n[:, 0:10, :], m8[qb + 1, :, 0:10, :]
                                )
                            else:
                                h0 = mpool.tile([128, 10, LB], F16, tag="mpk")
                                nc.sync.dma_start(
                                    h0[:], mu8[qb + 1, :, 0:10, :]
                                )
                                m_next = ("f16", [h0, None])
                        if g == 8:
                            if m_next[0] == "f8":
                                nc.sync.dma_start(
                                    m_next[1][:, 10:C1, :], m8[qb + 1, :, 10:C1, :]
                                )
                            else:
                                h1 = mpool.tile([128, 10, LB], F16, tag="mpk")
                                nc.sync.dma_start(h1[:], mu8[qb + 1, :, 10:C1, :])
                                m_next[1][1] = h1
                # park the pass-1 partial (O^T + Z rows) in SBUF
                nc.vector.tensor_copy(
                    part_sb[:, qb, :], p_o[0 : H + 1, :]
                )
                cur_mask = m_next

            # ---- pass 2: chunks 24:32 for every q-block ----
            mt2 = kv_tail.pop("mp2")
            for qb in range(NQB):
                qs = qb * LB
                p_o = ps_o.tile([128, LB], F32, tag="p_o")
                for g in range(NG2):
                    cA = C1 + 2 * g
                    ps = scores_and_exp(qb, cA)
                    msrc = mt2[:, 2 * g : 2 * g + 2, :]
                    exp_mask_av(ps, p_o, cA, msrc, g == 0, g == NG2 - 1)
                    if qb + 1 < NQB and g == 1:
                        m2n = mpool2.tile([128, NCH - C1, LB], F16, tag="mp2")
                        nc.sync.dma_start(m2n[:], mu8[qb + 1, :, C1:NCH, :])
                # epilogue: add pass-1 partial, ship unnormalized O^T + Z
                o_sb = opool.tile([H + 1, LB], F32, tag="osb")
                nc.vector.tensor_add(
                    o_sb[:], part_sb[:, qb, :], p_o[0 : H + 1, :]
                )
                nc.sync.dma_start(out[:, qs : qs + LB], o_sb[:])
                if qb + 1 < NQB:
                    mt2 = m2n
    nc.compile()
    return nc


_NC_CACHE = {}


def _shuffle_pcl(xT):
    """xT: [E, L] -> [NQB, 128, NEC, LB]."""
    a = xT.reshape(NEC, 128, NQB, LB)
    return np.ascontiguousarray(a.transpose(2, 1, 0, 3))


def _shuffle_mask(forb_b):
    """forb_b: [L, L] bool (True = forbidden) -> [NQB, 128, NCH, LB] u8
    allowed mask: [qb, p, c, q'] = 1 - forb[qb*512+q', c*128+p]."""
    A = forb_b.T.reshape(NCH, 128, NQB, LB)
    return (1 - np.ascontiguousarray(A.transpose(2, 1, 0, 3))).astype(np.uint8)


def kernel(query, key, value, mask, WQ, WK, WV):
    if "nc" not in _NC_CACHE:
        _NC_CACHE["nc"] = build_nc()
    nc = _NC_CACHE["nc"]

    wqT = np.asarray(WQ, dtype=np.float16).T  # [E, H]
    wkT = np.asarray(WK, dtype=np.float16).T
    wvT = np.ascontiguousarray(np.asarray(WV, dtype=np.float16).T)
    wqD = np.ascontiguousarray(np.concatenate([wqT, wqT], axis=1))
    wkD = np.ascontiguousarray(np.concatenate([wkT, wkT], axis=1))
    idn = np.eye(64, dtype=np.float16)
    forb = np.asarray(mask)  # [B, L, L], True where forbidden
    in_maps = []
    for b in range(B):
        allow = _shuffle_mask(forb[b])  # [NQB, 128, NCH, LB] u8 {0,1}
        m8b = np.where(
            allow[:NQB8, :, :C1, :] > 0, np.uint8(0x38), np.uint8(0)
        ).view(ml_dtypes.float8_e4m3)
        in_maps.append(
            {
                "q2": _shuffle_pcl(np.asarray(query[b], dtype=np.float16).T),
                "k2": _shuffle_pcl(np.asarray(key[b], dtype=np.float16).T),
                "v2": _shuffle_pcl(np.asarray(value[b], dtype=np.float16).T),
                "m8": np.ascontiguousarray(m8b),
                "mu8": allow.astype(np.float16),
                "wqD": wqD,
                "wkD": wkD,
                "wvT": wvT,
                "ident": idn,
            }
        )
    res = bass_utils.run_bass_kernel_spmd(nc, in_maps, core_ids=list(range(NCORES)))
    outs = []
    for b in range(B):
        ot = res.results[b]["out"].astype(np.float64)  # [65, L]
        o = (ot[0:H] / ot[H : H + 1]).T  # [L, H]
        outs.append(o.astype(np.float32))
    return np.stack(outs, axis=0)


if __name__ == "__main__":
    rng = np.random.default_rng(0)
    q = rng.standard_normal((B, L, E), dtype=np.float32)
    k = rng.standard_normal((B, L, E), dtype=np.float32)
    v = rng.standard_normal((B, L, E), dtype=np.float32)
    m = rng.integers(0, 2, size=(B, L, L)).astype(bool)
    s = 1.0 / np.sqrt(E)
    wq = rng.uniform(-s, s, size=(H, E)).astype(np.float32)
    wk = rng.uniform(-s, s, size=(H, E)).astype(np.float32)
    wv = rng.uniform(-s, s, size=(H, E)).astype(np.float32)
    o = kernel(query=q, key=k, value=v, mask=m, WQ=wq, WK=wk, WV=wv)
    print(o.shape, o.dtype)
